# revision 71
# baseline (speedup 1.0000x reference)
"""BiDet VGG block (binary conv + sync-BN + residual) on 8 TRN2 NeuronCores.

Data-parallel: 4 images per core; 3x3 binarized conv as 9 shifted DoubleRow-fp8
matmuls (sign values are exact in fp8); BN batch stats via bn_stats + a 2KB
AllReduce; normalize + residual fused into a vector tensor_scalar + add pass.
"""

import sys

sys.path.insert(0, "/opt/trn_rl_repo")

import numpy as np

import concourse.bass as bass
import concourse.bacc as bacc
import concourse.mybir as mybir
import concourse.tile as tile
from concourse.bass_utils import run_bass_kernel_spmd

B, C, H, W = 32, 256, 56, 56
NCORES = 8
BL = B // NCORES            # images per core
P = 128
CB = C // P                 # channel blocks
IMG = H * W                 # 3136
PW = W + 2                  # padded row width 58
S2 = 3392                   # per-(img, cblk) padded image region (16B aligned, >= 58*58)
NT = 7                      # row-tiles per image (8 rows each)
TIL = 8 * PW                # 464 psum columns per tile (incl. 2 garbage cols/row)
VAL = 8 * W                 # 448 valid columns per tile
N_LOC = BL * IMG            # elems per channel per core
N_GLOB = B * IMG            # true elems per channel globally
BN_EPS = 1e-5

F32 = mybir.dt.float32
BF16 = mybir.dt.bfloat16
FP8 = mybir.dt.float8e4

_CACHE = {}


def _r3(ap, h, w):
    return ap.rearrange("p (h w) -> p h w", h=h, w=w)


def build_nc(m_reps=1, skip_ar=False, n_taps=9):
    nc = bacc.Bacc("TRN2", target_bir_lowering=False, debug=False, num_devices=NCORES)

    xbf_t = nc.dram_tensor("xbf", [BL, C, H, W], BF16, kind="ExternalInput")
    w_t = nc.dram_tensor("weight", [C, C, 3, 3], F32, kind="ExternalInput")
    g_t = nc.dram_tensor("gamma", [C], F32, kind="ExternalInput")
    b_t = nc.dram_tensor("beta", [C], F32, kind="ExternalInput")
    o_t = nc.dram_tensor("out", [BL, C, H, W], BF16, kind="ExternalOutput")

    ident_d = nc.inline_tensor(np.eye(P).astype(mybir.dt.np(BF16)), name="ident_bf16")

    with tile.TileContext(nc) as tc:
        with (
            tc.tile_pool(name="persist", bufs=1) as pp,
            tc.tile_pool(name="wnat", bufs=2) as wnat_pool,
            tc.tile_pool(name="wsign", bufs=2) as wsign_pool,
            tc.tile_pool(name="xin", bufs=8) as xin_pool,
            tc.tile_pool(name="t1", bufs=2) as t1_pool,
            tc.tile_pool(name="oput", bufs=3) as oput_pool,
            tc.tile_pool(name="dram", bufs=1, space="DRAM") as dram_pool,
        ):
            # persistent tiles
            ident = pp.tile([P, P], BF16, name="ident")
            wsb = pp.tile([P, CB, 9, CB, P], FP8, name="wsb")      # lhsT weights
            # padded sign(x)/2, 2-image ring as SEPARATE tiles so Tile's
            # whole-tile dependency tracking doesn't serialize sign(k+1)
            # against conv(k)'s reads of the other slot
            xbs = [
                pp.tile([P, CB, S2], FP8, name=f"xb{r}", uniquify=False)
                for r in range(2)
            ]
            ybuf = pp.tile([P, CB, BL, IMG], BF16, name="ybuf")    # conv output
            statsb = pp.tile([P, CB, BL * NT, 6], F32, name="statsb")
            gamma_sb = pp.tile([P, CB], F32, name="gamma_sb")
            beta_sb = pp.tile([P, CB], F32, name="beta_sb")
            loc_ms = pp.tile([P, CB, 2], F32, name="loc_ms")       # (mean,var) per cb
            ar_in = pp.tile([P, 4], F32, name="ar_in")
            g_stats = pp.tile([P, 4], F32, name="g_stats")
            s2_sb = pp.tile([P, CB], F32, name="s2_sb")
            bp_sb = pp.tile([P, CB], F32, name="bp_sb")
            gsc = pp.tile([P, 4], F32, name="gsc")
            gamma2_sb = pp.tile([P, CB], F32, name="gamma2_sb")
            tmp_a = pp.tile([P, CB], F32, name="tmp_a")
            tmp_b = pp.tile([P, CB], F32, name="tmp_b")
            tmp_c = pp.tile([P, CB], F32, name="tmp_c")
            eps_sb = pp.tile([P, 1], F32, name="eps_sb")
            warm_sb = pp.tile([P, 1], F32, name="warm_sb")
            nc.vector.memset(eps_sb[:], BN_EPS)
            # pre-load the sqrt activation table set off the critical path
            nc.scalar.activation(
                warm_sb[:], eps_sb[:], mybir.ActivationFunctionType.Sqrt,
                bias=eps_sb[:],
            )

            # zero the padded-input rings first (borders stay zero forever)
            for r in range(2):
                nc.vector.memset(xbs[r][:, :, :].bitcast(mybir.dt.uint32), 0)

            def emit_sign(img, eng, split_queues=False):
                # xb' = (x > 0) - 0.5 = sign(x)/2 — exact in fp8; the 2x is
                # folded into the BN sums and the phase-2 scale.
                ring = img % 2
                for cb in range(CB):
                    xin = xin_pool.tile([P, IMG], BF16, name="xin")
                    dma_eng = nc.gpsimd if (split_queues and cb == 1) else nc.sync
                    dma_eng.dma_start(
                        out=xin[:],
                        in_=xbf_t.ap()[img, cb * P : (cb + 1) * P, :, :].rearrange(
                            "p h w -> p (h w)"
                        ),
                    )
                    dst = _r3(xbs[ring][:, cb, PW + 1 : PW + 1 + H * PW], H, PW)
                    eng.tensor_scalar(
                        dst[:, :, :W],
                        _r3(xin[:], H, W),
                        0.0,
                        0.5,
                        mybir.AluOpType.is_gt,
                        mybir.AluOpType.subtract,
                    )

            # image 0 gates the first conv group — issue its DMAs (on both
            # queues, in parallel) + binarize on the fast DVE before the
            # weight loads hit the queues.
            emit_sign(0, nc.vector, split_queues=True)

            nc.sync.dma_start(out=ident[:], in_=ident_d.ap())
            with tc.tile_pool(name="psw", bufs=4, space="PSUM") as psw_pool:
                for cb_o in range(CB):
                    wnat = wnat_pool.tile([P, C * 9], F32, name="wnat")
                    # SWDGE queue: don't serialize behind image-0's input DMAs
                    nc.gpsimd.dma_start(
                        out=wnat[:],
                        in_=w_t.ap()[cb_o * P : (cb_o + 1) * P, :, :, :].rearrange(
                            "p a b c -> p (a b c)"
                        ),
                    )
                    wnat3 = wnat.rearrange("p (ci t) -> p ci t", t=9)
                    # sign+transpose in tap-chunks (separate tiles) so the
                    # first taps' lhsT weights are ready before the rest
                    for t0, t1 in ((0, 3), (3, 9)):
                        wsgc = wsign_pool.tile([P, C, t1 - t0], BF16, name="wsig")
                        nc.scalar.sign(wsgc[:], wnat3[:, :, t0:t1])
                        for tap in range(t0, t1):
                            for cit in range(CB):
                                ptw = psw_pool.tile([P, P], BF16, name="ptw")
                                nc.tensor.transpose(
                                    ptw[:],
                                    wsgc[:, cit * P : (cit + 1) * P, tap - t0],
                                    ident[:],
                                )
                                nc.scalar.copy(wsb[:, cb_o, tap, cit, :], ptw[:])

            g2 = g_t.ap().rearrange("(a b) -> a b", b=1)
            b2 = b_t.ap().rearrange("(a b) -> a b", b=1)
            for cb in range(CB):
                nc.sync.dma_start(
                    out=gamma_sb[:, cb : cb + 1], in_=g2[cb * P : (cb + 1) * P, :]
                )
                nc.sync.dma_start(
                    out=beta_sb[:, cb : cb + 1], in_=b2[cb * P : (cb + 1) * P, :]
                )
            # 2*gamma precomputed off the critical path (y is stored as y/2)
            nc.scalar.mul(gamma2_sb[:, :], gamma_sb[:, :], 2.0)

            for _rep in range(m_reps):
                # ---- phase 1: sign(x), conv, drain+stats ----
                with tc.tile_pool(name="psc", bufs=8, space="PSUM") as psc_pool:
                    for img in range(BL):
                        ring = img % 2
                        if not (_rep == 0 and img == 0):
                            emit_sign(img, nc.vector)

                        for cb_o in range(CB):
                            lhs_all = wsb[:, cb_o, :, :, :]
                            for group in (range(0, 4), range(4, NT)):
                                pts = [
                                    psc_pool.tile([P, TIL], F32, name="ptc")
                                    for _ in group
                                ]
                                for tap in range(n_taps):
                                    kh, kw = tap // 3, tap % 3
                                    lhs = lhs_all[:, tap, :, :]
                                    for ti, t in enumerate(group):
                                        off = (t * 8 + kh) * PW + kw
                                        rhs = xbs[ring][:, :, off : off + TIL]
                                        nc.tensor.matmul(
                                            pts[ti][:],
                                            lhs,
                                            rhs,
                                            start=(tap == 0),
                                            stop=(tap == n_taps - 1),
                                            perf_mode=mybir.MatmulPerfMode.DoubleRow,
                                            skip_group_check=True,
                                        )
                                for ti, t in enumerate(group):
                                    pt3 = _r3(pts[ti][:], 8, PW)[:, :, :W]
                                    ysl = ybuf[:, cb_o, img, t * VAL : (t + 1) * VAL]
                                    nc.scalar.activation(
                                        _r3(ysl, 8, W), pt3,
                                        mybir.ActivationFunctionType.Copy,
                                    )
                                    # stats from the dense drained strip (no garbage)
                                    nc.vector.bn_stats(
                                        statsb[:, cb_o, img * NT + t, :], ysl
                                    )

                # prefetch all phase-2 residual inputs now — before anything
                # that waits on the collective lands in the DMA queues
                p2_xins = []
                for img in range(BL):
                    for cb in range(CB):
                        xin = xin_pool.tile([P, IMG], BF16, name="xin")
                        nc.sync.dma_start(
                            out=xin[:],
                            in_=xbf_t.ap()[img, cb * P : (cb + 1) * P, :, :].rearrange(
                                "p h w -> p (h w)"
                            ),
                        )
                        p2_xins.append(xin)

                # ---- local aggregation -> (sum, sumsq), both cb at once ----
                for cb in range(CB):
                    nc.vector.bn_aggr(loc_ms[:, cb, :], statsb[:, cb, :, :])
                means = loc_ms[:, :, 0]   # [P, CB] step 2
                vars_ = loc_ms[:, :, 1]
                ar2 = ar_in.rearrange("p (a b) -> p a b", b=2)
                # stats were computed on y/2: sums scale by 2x and 4x exactly
                nc.scalar.square(tmp_a[:, :], means)
                nc.vector.tensor_add(tmp_b[:, :], vars_, tmp_a[:, :])
                nc.scalar.mul(ar2[:, :, 0], means, float(2 * N_LOC))
                nc.scalar.mul(ar2[:, :, 1], tmp_b[:, :], float(4 * N_LOC))

                cci = dram_pool.tile([P, 4], F32, name="cc_in")
                cco = dram_pool.tile([P, 4], F32, name="cc_out")
                nc.sync.dma_start(out=cci[:], in_=ar_in[:])
                if skip_ar:
                    nc.sync.dma_start(out=cco[:], in_=cci[:])
                else:
                    nc.gpsimd.collective_compute(
                        "AllReduce",
                        mybir.AluOpType.add,
                        replica_groups=[list(range(NCORES))],
                        ins=[cci.opt()],
                        outs=[cco.opt()],
                    )
                # readback on the sync queue is safe now: every DMA emitted
                # after it (the phase-2 stores) is collective-dependent anyway
                nc.sync.dma_start(out=g_stats[:], in_=cco[:])

                # ---- global scale/bias: s2 = 2*gamma*rsqrt(var+eps) (stored y
                # is y/2), b' = beta - mean*s.  Minimal post-collective chain.
                inv_n = 1.0 / float(N_GLOB)
                nc.scalar.mul(gsc[:, :], g_stats[:, :], inv_n)     # means+ex2 at once
                gs2 = gsc.rearrange("p (a b) -> p a b", b=2)
                gmean = gs2[:, :, 0]
                gex2 = gs2[:, :, 1]
                nc.scalar.square(tmp_a[:, :], gmean)
                nc.vector.tensor_sub(tmp_b[:, :], gex2, tmp_a[:, :])   # var
                nc.scalar.activation(
                    tmp_c[:, :], tmp_b[:, :], mybir.ActivationFunctionType.Sqrt,
                    bias=eps_sb[:],
                )
                nc.vector.reciprocal(tmp_a[:, :], tmp_c[:, :])     # 1/sqrt(var+eps)
                nc.vector.tensor_mul(s2_sb[:, :], tmp_a[:, :], gamma2_sb[:, :])
                nc.vector.tensor_mul(tmp_b[:, :], gmean, s2_sb[:, :])  # 2*mean*s
                # b' = beta - 0.5*(2*mean*s)
                nc.vector.scalar_tensor_tensor(
                    bp_sb[:, :], tmp_b[:, :], -0.5, beta_sb[:, :],
                    mybir.AluOpType.mult, mybir.AluOpType.add,
                )

                # ---- phase 2: out = y*s + b' + x (vector-only pipeline) ----
                for img in range(BL):
                    for cb in range(CB):
                        k = img * CB + cb
                        xin = p2_xins[k]
                        t1 = t1_pool.tile([P, IMG], BF16, name="t1")
                        if k in (1, 3, 5):
                            # run some scale+bias passes on the otherwise-idle
                            # ScalarE so the DVE pipe only carries the adds
                            nc.scalar.activation(
                                t1[:],
                                ybuf[:, cb, img, :],
                                mybir.ActivationFunctionType.Identity,
                                bias=bp_sb[:, cb : cb + 1],
                                scale=s2_sb[:, cb : cb + 1],
                            )
                        else:
                            nc.vector.tensor_scalar(
                                t1[:],
                                ybuf[:, cb, img, :],
                                s2_sb[:, cb : cb + 1],
                                bp_sb[:, cb : cb + 1],
                                mybir.AluOpType.mult,
                                mybir.AluOpType.add,
                            )
                        oput = oput_pool.tile([P, IMG], BF16, name="oput")
                        nc.vector.tensor_add(oput[:], t1[:], xin[:])
                        nc.sync.dma_start(
                            out=o_t.ap()[img, cb * P : (cb + 1) * P, :, :].rearrange(
                                "p h w -> p (h w)"
                            ),
                            in_=oput[:],
                        )

    nc.compile()
    return nc


def get_nc():
    if "nc" not in _CACHE:
        _CACHE["nc"] = build_nc()
    return _CACHE["nc"]


def kernel(x, weight, gamma, beta, _want_results=False, **run_kwargs):
    nc = get_nc()
    bf16 = mybir.dt.np(BF16)
    in_maps = []
    for core in range(NCORES):
        shard = np.ascontiguousarray(x[core * BL : (core + 1) * BL]).astype(
            np.float32, copy=False
        )
        in_maps.append(
            {
                "xbf": shard.astype(bf16),
                "weight": np.ascontiguousarray(weight).astype(np.float32, copy=False),
                "gamma": np.ascontiguousarray(gamma).astype(np.float32, copy=False),
                "beta": np.ascontiguousarray(beta).astype(np.float32, copy=False),
            }
        )
    res = run_bass_kernel_spmd(nc, in_maps, core_ids=list(range(NCORES)), **run_kwargs)
    out = np.concatenate(
        [
            np.asarray(res.results[c]["out"])
            .astype(np.float32)
            .reshape(BL, C, H, W)
            for c in range(NCORES)
        ],
        axis=0,
    )
    if _want_results:
        return out, res
    return out


# revision 77
# speedup vs baseline: 1.0142x; 1.0142x over previous
"""BiDet VGG block (binary conv + sync-BN + residual) on 8 TRN2 NeuronCores.

Data-parallel: 4 images per core; 3x3 binarized conv as 9 shifted DoubleRow-fp8
matmuls (sign values are exact in fp8); BN batch stats via bn_stats + a 2KB
AllReduce; normalize + residual fused into a vector tensor_scalar + add pass.
"""

import sys

sys.path.insert(0, "/opt/trn_rl_repo")

import numpy as np

import concourse.bass as bass
import concourse.bacc as bacc
import concourse.mybir as mybir
import concourse.tile as tile
from concourse.bass_utils import run_bass_kernel_spmd

B, C, H, W = 32, 256, 56, 56
NCORES = 8
BL = B // NCORES            # images per core
P = 128
CB = C // P                 # channel blocks
IMG = H * W                 # 3136
PW = W + 2                  # padded row width 58
S2 = 3392                   # per-(img, cblk) padded image region (16B aligned, >= 58*58)
NT = 7                      # row-tiles per image (8 rows each)
TIL = 8 * PW                # 464 psum columns per tile (incl. 2 garbage cols/row)
VAL = 8 * W                 # 448 valid columns per tile
N_LOC = BL * IMG            # elems per channel per core
N_GLOB = B * IMG            # true elems per channel globally
BN_EPS = 1e-5

F32 = mybir.dt.float32
BF16 = mybir.dt.bfloat16
FP8 = mybir.dt.float8e4

_CACHE = {}


def _r3(ap, h, w):
    return ap.rearrange("p (h w) -> p h w", h=h, w=w)


def build_nc(m_reps=1, skip_ar=False, n_taps=9):
    nc = bacc.Bacc("TRN2", target_bir_lowering=False, debug=False, num_devices=NCORES)

    xbf_t = nc.dram_tensor("xbf", [BL, C, H, W], BF16, kind="ExternalInput")
    w_t = nc.dram_tensor("weight", [C, C, 3, 3], F32, kind="ExternalInput")
    g_t = nc.dram_tensor("gamma", [C], F32, kind="ExternalInput")
    b_t = nc.dram_tensor("beta", [C], F32, kind="ExternalInput")
    o_t = nc.dram_tensor("out", [BL, C, H, W], BF16, kind="ExternalOutput")

    ident_d = nc.inline_tensor(np.eye(P).astype(mybir.dt.np(BF16)), name="ident_bf16")

    with tile.TileContext(nc) as tc:
        with (
            tc.tile_pool(name="persist", bufs=1) as pp,
            tc.tile_pool(name="wnat", bufs=2) as wnat_pool,
            tc.tile_pool(name="wsign", bufs=2) as wsign_pool,
            tc.tile_pool(name="xin", bufs=8) as xin_pool,
            tc.tile_pool(name="t1", bufs=2) as t1_pool,
            tc.tile_pool(name="oput", bufs=3) as oput_pool,
            tc.tile_pool(name="dram", bufs=1, space="DRAM") as dram_pool,
        ):
            # persistent tiles
            ident = pp.tile([P, P], BF16, name="ident")
            wsb = pp.tile([P, CB, 9, CB, P], FP8, name="wsb")      # lhsT weights
            # padded sign(x)/2, 2-image ring as SEPARATE tiles so Tile's
            # whole-tile dependency tracking doesn't serialize sign(k+1)
            # against conv(k)'s reads of the other slot
            xbs = [
                pp.tile([P, CB, S2], FP8, name=f"xb{r}", uniquify=False)
                for r in range(2)
            ]
            ybuf = pp.tile([P, CB, BL, IMG], BF16, name="ybuf")    # conv output
            statsb = pp.tile([P, CB, BL * NT, 6], F32, name="statsb")
            gamma_sb = pp.tile([P, CB], F32, name="gamma_sb")
            beta_sb = pp.tile([P, CB], F32, name="beta_sb")
            loc_ms = pp.tile([P, CB, 2], F32, name="loc_ms")       # (mean,var) per cb
            ar_in = pp.tile([P, 4], F32, name="ar_in")
            g_stats = pp.tile([P, 4], F32, name="g_stats")
            s2_sb = pp.tile([P, CB], F32, name="s2_sb")
            bp_sb = pp.tile([P, CB], F32, name="bp_sb")
            gsc = pp.tile([P, 4], F32, name="gsc")
            gamma2_sb = pp.tile([P, CB], F32, name="gamma2_sb")
            tmp_a = pp.tile([P, CB], F32, name="tmp_a")
            tmp_b = pp.tile([P, CB], F32, name="tmp_b")
            tmp_c = pp.tile([P, CB], F32, name="tmp_c")
            eps_sb = pp.tile([P, 1], F32, name="eps_sb")
            warm_sb = pp.tile([P, 1], F32, name="warm_sb")
            nc.vector.memset(eps_sb[:], BN_EPS)
            # pre-load the sqrt activation table set off the critical path
            nc.scalar.activation(
                warm_sb[:], eps_sb[:], mybir.ActivationFunctionType.Sqrt,
                bias=eps_sb[:],
            )

            # zero the padded-input rings first (borders stay zero forever)
            for r in range(2):
                nc.vector.memset(xbs[r][:, :, :].bitcast(mybir.dt.uint32), 0)

            def emit_sign(img, eng, split_queues=False):
                # xb' = (x > 0) - 0.5 = sign(x)/2 — exact in fp8; the 2x is
                # folded into the BN sums and the phase-2 scale.
                ring = img % 2
                for cb in range(CB):
                    xin = xin_pool.tile([P, IMG], BF16, name="xin")
                    dma_eng = nc.gpsimd if (split_queues and cb == 1) else nc.sync
                    dma_eng.dma_start(
                        out=xin[:],
                        in_=xbf_t.ap()[img, cb * P : (cb + 1) * P, :, :].rearrange(
                            "p h w -> p (h w)"
                        ),
                    )
                    dst = _r3(xbs[ring][:, cb, PW + 1 : PW + 1 + H * PW], H, PW)
                    eng.tensor_scalar(
                        dst[:, :, :W],
                        _r3(xin[:], H, W),
                        0.0,
                        0.5,
                        mybir.AluOpType.is_gt,
                        mybir.AluOpType.subtract,
                    )

            # image 0 gates the first conv group — issue its DMAs (on both
            # queues, in parallel) + binarize on the fast DVE before the
            # weight loads hit the queues.
            emit_sign(0, nc.vector, split_queues=True)

            nc.sync.dma_start(out=ident[:], in_=ident_d.ap())
            with tc.tile_pool(name="psw", bufs=8, space="PSUM") as psw_pool:
                for cb_o in range(CB):
                    wnat = wnat_pool.tile([P, C * 9], F32, name="wnat")
                    # alternate queues so the two loads run in parallel and
                    # don't serialize behind image-0's input DMAs
                    wdma = nc.gpsimd if cb_o == 0 else nc.sync
                    wdma.dma_start(
                        out=wnat[:],
                        in_=w_t.ap()[cb_o * P : (cb_o + 1) * P, :, :, :].rearrange(
                            "p a b c -> p (a b c)"
                        ),
                    )
                    wnat3 = wnat.rearrange("p (ci t) -> p ci t", t=9)
                    # binarize weights as (w>0)-0.5 = sign(w)/2 on DVE (keeps
                    # ScalarE free for the transpose drains); products are
                    # then y/4 — folded exactly into the BN sums and scale.
                    # Chunked by taps (separate tiles) so the first taps'
                    # lhsT weights are ready before the rest.
                    for t0, t1 in ((0, 3), (3, 9)):
                        wsgc = wsign_pool.tile([P, C, t1 - t0], BF16, name="wsig")
                        nc.vector.tensor_scalar(
                            wsgc[:],
                            wnat3[:, :, t0:t1],
                            0.0,
                            0.5,
                            mybir.AluOpType.is_gt,
                            mybir.AluOpType.subtract,
                        )
                        for tap in range(t0, t1):
                            for cit in range(CB):
                                ptw = psw_pool.tile([P, P], BF16, name="ptw")
                                nc.tensor.transpose(
                                    ptw[:],
                                    wsgc[:, cit * P : (cit + 1) * P, tap - t0],
                                    ident[:],
                                )
                                nc.scalar.copy(wsb[:, cb_o, tap, cit, :], ptw[:])

            g2 = g_t.ap().rearrange("(a b) -> a b", b=1)
            b2 = b_t.ap().rearrange("(a b) -> a b", b=1)
            for cb in range(CB):
                nc.sync.dma_start(
                    out=gamma_sb[:, cb : cb + 1], in_=g2[cb * P : (cb + 1) * P, :]
                )
                nc.sync.dma_start(
                    out=beta_sb[:, cb : cb + 1], in_=b2[cb * P : (cb + 1) * P, :]
                )
            # 4*gamma precomputed off the critical path (y is stored as y/4)
            nc.scalar.mul(gamma2_sb[:, :], gamma_sb[:, :], 4.0)

            for _rep in range(m_reps):
                # ---- phase 1: sign(x), conv, drain+stats ----
                with tc.tile_pool(name="psc", bufs=8, space="PSUM") as psc_pool:
                    for img in range(BL):
                        ring = img % 2
                        if not (_rep == 0 and img == 0):
                            emit_sign(img, nc.vector)

                        for cb_o in range(CB):
                            lhs_all = wsb[:, cb_o, :, :, :]
                            for group in (range(0, 4), range(4, NT)):
                                pts = [
                                    psc_pool.tile([P, TIL], F32, name="ptc")
                                    for _ in group
                                ]
                                for tap in range(n_taps):
                                    kh, kw = tap // 3, tap % 3
                                    lhs = lhs_all[:, tap, :, :]
                                    for ti, t in enumerate(group):
                                        off = (t * 8 + kh) * PW + kw
                                        rhs = xbs[ring][:, :, off : off + TIL]
                                        nc.tensor.matmul(
                                            pts[ti][:],
                                            lhs,
                                            rhs,
                                            start=(tap == 0),
                                            stop=(tap == n_taps - 1),
                                            perf_mode=mybir.MatmulPerfMode.DoubleRow,
                                            skip_group_check=True,
                                        )
                                for ti, t in enumerate(group):
                                    pt3 = _r3(pts[ti][:], 8, PW)[:, :, :W]
                                    ysl = ybuf[:, cb_o, img, t * VAL : (t + 1) * VAL]
                                    nc.scalar.activation(
                                        _r3(ysl, 8, W), pt3,
                                        mybir.ActivationFunctionType.Copy,
                                    )
                                    # stats from the dense drained strip (no garbage)
                                    nc.vector.bn_stats(
                                        statsb[:, cb_o, img * NT + t, :], ysl
                                    )

                # prefetch all phase-2 residual inputs now — before anything
                # that waits on the collective lands in the DMA queues
                p2_xins = []
                for img in range(BL):
                    for cb in range(CB):
                        xin = xin_pool.tile([P, IMG], BF16, name="xin")
                        nc.sync.dma_start(
                            out=xin[:],
                            in_=xbf_t.ap()[img, cb * P : (cb + 1) * P, :, :].rearrange(
                                "p h w -> p (h w)"
                            ),
                        )
                        p2_xins.append(xin)

                # ---- local aggregation -> (sum, sumsq), both cb at once ----
                for cb in range(CB):
                    nc.vector.bn_aggr(loc_ms[:, cb, :], statsb[:, cb, :, :])
                means = loc_ms[:, :, 0]   # [P, CB] step 2
                vars_ = loc_ms[:, :, 1]
                ar2 = ar_in.rearrange("p (a b) -> p a b", b=2)
                # stats were computed on y/4: sums scale by 4x and 16x exactly
                nc.scalar.square(tmp_a[:, :], means)
                nc.vector.tensor_add(tmp_b[:, :], vars_, tmp_a[:, :])
                nc.scalar.mul(ar2[:, :, 0], means, float(4 * N_LOC))
                nc.scalar.mul(ar2[:, :, 1], tmp_b[:, :], float(16 * N_LOC))

                cci = dram_pool.tile([P, 4], F32, name="cc_in")
                cco = dram_pool.tile([P, 4], F32, name="cc_out")
                nc.sync.dma_start(out=cci[:], in_=ar_in[:])
                if skip_ar:
                    nc.sync.dma_start(out=cco[:], in_=cci[:])
                else:
                    nc.gpsimd.collective_compute(
                        "AllReduce",
                        mybir.AluOpType.add,
                        replica_groups=[list(range(NCORES))],
                        ins=[cci.opt()],
                        outs=[cco.opt()],
                    )
                # readback on the sync queue is safe now: every DMA emitted
                # after it (the phase-2 stores) is collective-dependent anyway
                nc.sync.dma_start(out=g_stats[:], in_=cco[:])

                # ---- global scale/bias: s2 = 4*gamma*rsqrt(var+eps) (stored y
                # is y/4), b' = beta - mean*s.  Minimal post-collective chain.
                inv_n = 1.0 / float(N_GLOB)
                nc.scalar.mul(gsc[:, :], g_stats[:, :], inv_n)     # means+ex2 at once
                gs2 = gsc.rearrange("p (a b) -> p a b", b=2)
                gmean = gs2[:, :, 0]
                gex2 = gs2[:, :, 1]
                nc.scalar.square(tmp_a[:, :], gmean)
                nc.vector.tensor_sub(tmp_b[:, :], gex2, tmp_a[:, :])   # var
                nc.scalar.activation(
                    tmp_c[:, :], tmp_b[:, :], mybir.ActivationFunctionType.Sqrt,
                    bias=eps_sb[:],
                )
                nc.vector.reciprocal(tmp_a[:, :], tmp_c[:, :])     # 1/sqrt(var+eps)
                nc.vector.tensor_mul(s2_sb[:, :], tmp_a[:, :], gamma2_sb[:, :])
                nc.vector.tensor_mul(tmp_b[:, :], gmean, s2_sb[:, :])  # 4*mean*s
                # b' = beta - 0.25*(4*mean*s)
                nc.vector.scalar_tensor_tensor(
                    bp_sb[:, :], tmp_b[:, :], -0.25, beta_sb[:, :],
                    mybir.AluOpType.mult, mybir.AluOpType.add,
                )

                # ---- phase 2: out = y*s + b' + x (vector-only pipeline) ----
                for img in range(BL):
                    for cb in range(CB):
                        k = img * CB + cb
                        xin = p2_xins[k]
                        t1 = t1_pool.tile([P, IMG], BF16, name="t1")
                        if k in (1, 3, 5):
                            # run some scale+bias passes on the otherwise-idle
                            # ScalarE so the DVE pipe only carries the adds
                            nc.scalar.activation(
                                t1[:],
                                ybuf[:, cb, img, :],
                                mybir.ActivationFunctionType.Identity,
                                bias=bp_sb[:, cb : cb + 1],
                                scale=s2_sb[:, cb : cb + 1],
                            )
                        else:
                            nc.vector.tensor_scalar(
                                t1[:],
                                ybuf[:, cb, img, :],
                                s2_sb[:, cb : cb + 1],
                                bp_sb[:, cb : cb + 1],
                                mybir.AluOpType.mult,
                                mybir.AluOpType.add,
                            )
                        oput = oput_pool.tile([P, IMG], BF16, name="oput")
                        nc.vector.tensor_add(oput[:], t1[:], xin[:])
                        nc.sync.dma_start(
                            out=o_t.ap()[img, cb * P : (cb + 1) * P, :, :].rearrange(
                                "p h w -> p (h w)"
                            ),
                            in_=oput[:],
                        )

    nc.compile()
    return nc


def get_nc():
    if "nc" not in _CACHE:
        _CACHE["nc"] = build_nc()
    return _CACHE["nc"]


def kernel(x, weight, gamma, beta, _want_results=False, **run_kwargs):
    nc = get_nc()
    bf16 = mybir.dt.np(BF16)
    in_maps = []
    for core in range(NCORES):
        shard = np.ascontiguousarray(x[core * BL : (core + 1) * BL]).astype(
            np.float32, copy=False
        )
        in_maps.append(
            {
                "xbf": shard.astype(bf16),
                "weight": np.ascontiguousarray(weight).astype(np.float32, copy=False),
                "gamma": np.ascontiguousarray(gamma).astype(np.float32, copy=False),
                "beta": np.ascontiguousarray(beta).astype(np.float32, copy=False),
            }
        )
    res = run_bass_kernel_spmd(nc, in_maps, core_ids=list(range(NCORES)), **run_kwargs)
    out = np.concatenate(
        [
            np.asarray(res.results[c]["out"])
            .astype(np.float32)
            .reshape(BL, C, H, W)
            for c in range(NCORES)
        ],
        axis=0,
    )
    if _want_results:
        return out, res
    return out


# revision 80
# speedup vs baseline: 1.0222x; 1.0079x over previous
"""BiDet VGG block (binary conv + sync-BN + residual) on 8 TRN2 NeuronCores.

Data-parallel: 4 images per core; 3x3 binarized conv as 9 shifted DoubleRow-fp8
matmuls (sign values are exact in fp8); BN batch stats via bn_stats + a 2KB
AllReduce; normalize + residual fused into a vector tensor_scalar + add pass.
"""

import sys

sys.path.insert(0, "/opt/trn_rl_repo")

import numpy as np

import concourse.bass as bass
import concourse.bacc as bacc
import concourse.mybir as mybir
import concourse.tile as tile
from concourse.bass_utils import run_bass_kernel_spmd

B, C, H, W = 32, 256, 56, 56
NCORES = 8
BL = B // NCORES            # images per core
P = 128
CB = C // P                 # channel blocks
IMG = H * W                 # 3136
PW = W + 2                  # padded row width 58
S2 = 3392                   # per-(img, cblk) padded image region (16B aligned, >= 58*58)
NT = 7                      # row-tiles per image (8 rows each)
TIL = 8 * PW                # 464 psum columns per tile (incl. 2 garbage cols/row)
VAL = 8 * W                 # 448 valid columns per tile
N_LOC = BL * IMG            # elems per channel per core
N_GLOB = B * IMG            # true elems per channel globally
BN_EPS = 1e-5

F32 = mybir.dt.float32
BF16 = mybir.dt.bfloat16
FP8 = mybir.dt.float8e4

_CACHE = {}


def _r3(ap, h, w):
    return ap.rearrange("p (h w) -> p h w", h=h, w=w)


def build_nc(m_reps=1, skip_ar=False, n_taps=9):
    nc = bacc.Bacc("TRN2", target_bir_lowering=False, debug=False, num_devices=NCORES)

    xbf_t = nc.dram_tensor("xbf", [BL, C, H, W], BF16, kind="ExternalInput")
    w_t = nc.dram_tensor("weight", [C, C, 3, 3], F32, kind="ExternalInput")
    g_t = nc.dram_tensor("gamma", [C], F32, kind="ExternalInput")
    b_t = nc.dram_tensor("beta", [C], F32, kind="ExternalInput")
    o_t = nc.dram_tensor("out", [BL, C, H, W], BF16, kind="ExternalOutput")

    ident_d = nc.inline_tensor(np.eye(P).astype(mybir.dt.np(BF16)), name="ident_bf16")

    with tile.TileContext(nc) as tc:
        with (
            tc.tile_pool(name="persist", bufs=1) as pp,
            tc.tile_pool(name="wnat", bufs=2) as wnat_pool,
            tc.tile_pool(name="wsign", bufs=2) as wsign_pool,
            tc.tile_pool(name="xin", bufs=8) as xin_pool,
            tc.tile_pool(name="t1", bufs=2) as t1_pool,
            tc.tile_pool(name="oput", bufs=3) as oput_pool,
            tc.tile_pool(name="dram", bufs=1, space="DRAM") as dram_pool,
        ):
            # persistent tiles
            ident = pp.tile([P, P], BF16, name="ident")
            wsb = pp.tile([P, CB, 9, CB, P], FP8, name="wsb")      # lhsT weights
            # padded sign(x)/2, 2-image ring as SEPARATE tiles so Tile's
            # whole-tile dependency tracking doesn't serialize sign(k+1)
            # against conv(k)'s reads of the other slot
            xbs = [
                pp.tile([P, CB, S2], FP8, name=f"xb{r}", uniquify=False)
                for r in range(2)
            ]
            ybuf = pp.tile([P, CB, BL, IMG], BF16, name="ybuf")    # conv output
            statsb = pp.tile([P, CB, BL * NT, 6], F32, name="statsb")
            gamma_sb = pp.tile([P, CB], F32, name="gamma_sb")
            beta_sb = pp.tile([P, CB], F32, name="beta_sb")
            loc_ms = pp.tile([P, CB, 2], F32, name="loc_ms")       # (mean,var) per cb
            ar_in = pp.tile([P, 4], F32, name="ar_in")
            g_stats = pp.tile([P, 4], F32, name="g_stats")
            s2_sb = pp.tile([P, CB], F32, name="s2_sb")
            bp_sb = pp.tile([P, CB], F32, name="bp_sb")
            gsc = pp.tile([P, 4], F32, name="gsc")
            gamma2_sb = pp.tile([P, CB], F32, name="gamma2_sb")
            tmp_a = pp.tile([P, CB], F32, name="tmp_a")
            tmp_b = pp.tile([P, CB], F32, name="tmp_b")
            tmp_c = pp.tile([P, CB], F32, name="tmp_c")
            eps_sb = pp.tile([P, 1], F32, name="eps_sb")
            warm_sb = pp.tile([P, 1], F32, name="warm_sb")
            nc.vector.memset(eps_sb[:], BN_EPS)
            # pre-load the sqrt activation table set off the critical path
            nc.scalar.activation(
                warm_sb[:], eps_sb[:], mybir.ActivationFunctionType.Sqrt,
                bias=eps_sb[:],
            )

            # zero the padded-input rings first (borders stay zero forever)
            for r in range(2):
                nc.vector.memset(xbs[r][:, :, :].bitcast(mybir.dt.uint32), 0)

            def emit_sign(img, eng, split_queues=False):
                # xb' = (x > 0) - 0.5 = sign(x)/2 — exact in fp8; the 2x is
                # folded into the BN sums and the phase-2 scale.
                ring = img % 2
                for cb in range(CB):
                    xin = xin_pool.tile([P, IMG], BF16, name="xin")
                    dma_eng = nc.gpsimd if (split_queues and cb == 1) else nc.sync
                    dma_eng.dma_start(
                        out=xin[:],
                        in_=xbf_t.ap()[img, cb * P : (cb + 1) * P, :, :].rearrange(
                            "p h w -> p (h w)"
                        ),
                    )
                    dst = _r3(xbs[ring][:, cb, PW + 1 : PW + 1 + H * PW], H, PW)
                    eng.tensor_scalar(
                        dst[:, :, :W],
                        _r3(xin[:], H, W),
                        0.0,
                        0.5,
                        mybir.AluOpType.is_gt,
                        mybir.AluOpType.subtract,
                    )

            # image 0 gates the first conv group — issue its DMAs (on both
            # queues, in parallel) + binarize on the fast DVE before the
            # weight loads hit the queues.
            emit_sign(0, nc.vector, split_queues=True)

            nc.sync.dma_start(out=ident[:], in_=ident_d.ap())
            with tc.tile_pool(name="psw", bufs=8, space="PSUM") as psw_pool:
                for cb_o in range(CB):
                    wnat = wnat_pool.tile([P, C * 9], F32, name="wnat")
                    # alternate queues so the two loads run in parallel and
                    # don't serialize behind image-0's input DMAs
                    wdma = nc.gpsimd if cb_o == 0 else nc.sync
                    wdma.dma_start(
                        out=wnat[:],
                        in_=w_t.ap()[cb_o * P : (cb_o + 1) * P, :, :, :].rearrange(
                            "p a b c -> p (a b c)"
                        ),
                    )
                    wnat3 = wnat.rearrange("p (ci t) -> p ci t", t=9)
                    # binarize weights as (w>0)-0.5 = sign(w)/2 on DVE (keeps
                    # ScalarE free for the transpose drains); products are
                    # then y/4 — folded exactly into the BN sums and scale.
                    # Chunked by taps (separate tiles) so the first taps'
                    # lhsT weights are ready before the rest.
                    for t0, t1 in ((0, 3), (3, 9)):
                        wsgc = wsign_pool.tile([P, C, t1 - t0], BF16, name="wsig")
                        nc.gpsimd.tensor_scalar(
                            wsgc[:],
                            wnat3[:, :, t0:t1],
                            0.0,
                            0.5,
                            mybir.AluOpType.is_gt,
                            mybir.AluOpType.subtract,
                        )
                        for tap in range(t0, t1):
                            for cit in range(CB):
                                ptw = psw_pool.tile([P, P], BF16, name="ptw")
                                nc.tensor.transpose(
                                    ptw[:],
                                    wsgc[:, cit * P : (cit + 1) * P, tap - t0],
                                    ident[:],
                                )
                                nc.scalar.copy(wsb[:, cb_o, tap, cit, :], ptw[:])

            g2 = g_t.ap().rearrange("(a b) -> a b", b=1)
            b2 = b_t.ap().rearrange("(a b) -> a b", b=1)
            for cb in range(CB):
                nc.sync.dma_start(
                    out=gamma_sb[:, cb : cb + 1], in_=g2[cb * P : (cb + 1) * P, :]
                )
                nc.sync.dma_start(
                    out=beta_sb[:, cb : cb + 1], in_=b2[cb * P : (cb + 1) * P, :]
                )
            # 4*gamma precomputed off the critical path (y is stored as y/4)
            nc.scalar.mul(gamma2_sb[:, :], gamma_sb[:, :], 4.0)

            for _rep in range(m_reps):
                # ---- phase 1: sign(x), conv, drain+stats ----
                with tc.tile_pool(name="psc", bufs=8, space="PSUM") as psc_pool:
                    for img in range(BL):
                        ring = img % 2
                        if not (_rep == 0 and img == 0):
                            emit_sign(img, nc.vector)

                        for cb_o in range(CB):
                            lhs_all = wsb[:, cb_o, :, :, :]
                            for group in (range(0, 4), range(4, NT)):
                                pts = [
                                    psc_pool.tile([P, TIL], F32, name="ptc")
                                    for _ in group
                                ]
                                for tap in range(n_taps):
                                    kh, kw = tap // 3, tap % 3
                                    lhs = lhs_all[:, tap, :, :]
                                    for ti, t in enumerate(group):
                                        off = (t * 8 + kh) * PW + kw
                                        rhs = xbs[ring][:, :, off : off + TIL]
                                        nc.tensor.matmul(
                                            pts[ti][:],
                                            lhs,
                                            rhs,
                                            start=(tap == 0),
                                            stop=(tap == n_taps - 1),
                                            perf_mode=mybir.MatmulPerfMode.DoubleRow,
                                            skip_group_check=True,
                                        )
                                for ti, t in enumerate(group):
                                    pt3 = _r3(pts[ti][:], 8, PW)[:, :, :W]
                                    ysl = ybuf[:, cb_o, img, t * VAL : (t + 1) * VAL]
                                    nc.scalar.activation(
                                        _r3(ysl, 8, W), pt3,
                                        mybir.ActivationFunctionType.Copy,
                                    )
                                    # stats from the dense drained strip (no garbage)
                                    nc.vector.bn_stats(
                                        statsb[:, cb_o, img * NT + t, :], ysl
                                    )

                # prefetch all phase-2 residual inputs now — before anything
                # that waits on the collective lands in the DMA queues
                p2_xins = []
                for img in range(BL):
                    for cb in range(CB):
                        xin = xin_pool.tile([P, IMG], BF16, name="xin")
                        nc.sync.dma_start(
                            out=xin[:],
                            in_=xbf_t.ap()[img, cb * P : (cb + 1) * P, :, :].rearrange(
                                "p h w -> p (h w)"
                            ),
                        )
                        p2_xins.append(xin)

                # ---- local aggregation -> (sum, sumsq), both cb at once ----
                for cb in range(CB):
                    nc.vector.bn_aggr(loc_ms[:, cb, :], statsb[:, cb, :, :])
                means = loc_ms[:, :, 0]   # [P, CB] step 2
                vars_ = loc_ms[:, :, 1]
                ar2 = ar_in.rearrange("p (a b) -> p a b", b=2)
                # stats were computed on y/4: sums scale by 4x and 16x exactly
                nc.scalar.square(tmp_a[:, :], means)
                nc.vector.tensor_add(tmp_b[:, :], vars_, tmp_a[:, :])
                nc.scalar.mul(ar2[:, :, 0], means, float(4 * N_LOC))
                nc.scalar.mul(ar2[:, :, 1], tmp_b[:, :], float(16 * N_LOC))

                cci = dram_pool.tile([P, 4], F32, name="cc_in")
                cco = dram_pool.tile([P, 4], F32, name="cc_out")
                nc.sync.dma_start(out=cci[:], in_=ar_in[:])
                if skip_ar:
                    nc.sync.dma_start(out=cco[:], in_=cci[:])
                else:
                    nc.gpsimd.collective_compute(
                        "AllReduce",
                        mybir.AluOpType.add,
                        replica_groups=[list(range(NCORES))],
                        ins=[cci.opt()],
                        outs=[cco.opt()],
                    )
                # readback on the sync queue is safe now: every DMA emitted
                # after it (the phase-2 stores) is collective-dependent anyway
                nc.sync.dma_start(out=g_stats[:], in_=cco[:])

                # ---- global scale/bias: s2 = 4*gamma*rsqrt(var+eps) (stored y
                # is y/4), b' = beta - mean*s.  Minimal post-collective chain.
                inv_n = 1.0 / float(N_GLOB)
                nc.scalar.mul(gsc[:, :], g_stats[:, :], inv_n)     # means+ex2 at once
                gs2 = gsc.rearrange("p (a b) -> p a b", b=2)
                gmean = gs2[:, :, 0]
                gex2 = gs2[:, :, 1]
                nc.scalar.square(tmp_a[:, :], gmean)
                nc.vector.tensor_sub(tmp_b[:, :], gex2, tmp_a[:, :])   # var
                nc.scalar.activation(
                    tmp_c[:, :], tmp_b[:, :], mybir.ActivationFunctionType.Sqrt,
                    bias=eps_sb[:],
                )
                nc.vector.reciprocal(tmp_a[:, :], tmp_c[:, :])     # 1/sqrt(var+eps)
                nc.vector.tensor_mul(s2_sb[:, :], tmp_a[:, :], gamma2_sb[:, :])
                nc.vector.tensor_mul(tmp_b[:, :], gmean, s2_sb[:, :])  # 4*mean*s
                # b' = beta - 0.25*(4*mean*s)
                nc.vector.scalar_tensor_tensor(
                    bp_sb[:, :], tmp_b[:, :], -0.25, beta_sb[:, :],
                    mybir.AluOpType.mult, mybir.AluOpType.add,
                )

                # ---- phase 2: out = y*s + b' + x (vector-only pipeline) ----
                for img in range(BL):
                    for cb in range(CB):
                        k = img * CB + cb
                        xin = p2_xins[k]
                        t1 = t1_pool.tile([P, IMG], BF16, name="t1")
                        if k in (1, 3, 5):
                            # run some scale+bias passes on the otherwise-idle
                            # ScalarE so the DVE pipe only carries the adds
                            nc.scalar.activation(
                                t1[:],
                                ybuf[:, cb, img, :],
                                mybir.ActivationFunctionType.Identity,
                                bias=bp_sb[:, cb : cb + 1],
                                scale=s2_sb[:, cb : cb + 1],
                            )
                        else:
                            nc.vector.tensor_scalar(
                                t1[:],
                                ybuf[:, cb, img, :],
                                s2_sb[:, cb : cb + 1],
                                bp_sb[:, cb : cb + 1],
                                mybir.AluOpType.mult,
                                mybir.AluOpType.add,
                            )
                        oput = oput_pool.tile([P, IMG], BF16, name="oput")
                        nc.vector.tensor_add(oput[:], t1[:], xin[:])
                        nc.sync.dma_start(
                            out=o_t.ap()[img, cb * P : (cb + 1) * P, :, :].rearrange(
                                "p h w -> p (h w)"
                            ),
                            in_=oput[:],
                        )

    nc.compile()
    return nc


def get_nc():
    if "nc" not in _CACHE:
        _CACHE["nc"] = build_nc()
    return _CACHE["nc"]


def kernel(x, weight, gamma, beta, _want_results=False, **run_kwargs):
    nc = get_nc()
    bf16 = mybir.dt.np(BF16)
    in_maps = []
    for core in range(NCORES):
        shard = np.ascontiguousarray(x[core * BL : (core + 1) * BL]).astype(
            np.float32, copy=False
        )
        in_maps.append(
            {
                "xbf": shard.astype(bf16),
                "weight": np.ascontiguousarray(weight).astype(np.float32, copy=False),
                "gamma": np.ascontiguousarray(gamma).astype(np.float32, copy=False),
                "beta": np.ascontiguousarray(beta).astype(np.float32, copy=False),
            }
        )
    res = run_bass_kernel_spmd(nc, in_maps, core_ids=list(range(NCORES)), **run_kwargs)
    out = np.concatenate(
        [
            np.asarray(res.results[c]["out"])
            .astype(np.float32)
            .reshape(BL, C, H, W)
            for c in range(NCORES)
        ],
        axis=0,
    )
    if _want_results:
        return out, res
    return out


# revision 91
# speedup vs baseline: 1.0382x; 1.0157x over previous
"""BiDet VGG block (binary conv + sync-BN + residual) on 8 TRN2 NeuronCores.

Data-parallel: 4 images per core; 3x3 binarized conv as 9 shifted DoubleRow-fp8
matmuls (sign values are exact in fp8); BN batch stats via bn_stats + a 2KB
AllReduce; normalize + residual fused into a vector tensor_scalar + add pass.
"""

import sys

sys.path.insert(0, "/opt/trn_rl_repo")

import numpy as np

import concourse.bass as bass
import concourse.bacc as bacc
import concourse.mybir as mybir
import concourse.tile as tile
from concourse.bass_utils import run_bass_kernel_spmd

B, C, H, W = 32, 256, 56, 56
NCORES = 8
BL = B // NCORES            # images per core
P = 128
CB = C // P                 # channel blocks
IMG = H * W                 # 3136
PW = W + 2                  # padded row width 58
S2 = 3392                   # per-(img, cblk) padded image region (16B aligned, >= 58*58)
NT = 7                      # row-tiles per image (8 rows each)
TIL = 8 * PW                # 464 psum columns per tile (incl. 2 garbage cols/row)
VAL = 8 * W                 # 448 valid columns per tile
N_LOC = BL * IMG            # elems per channel per core
N_GLOB = B * IMG            # true elems per channel globally
BN_EPS = 1e-5

F32 = mybir.dt.float32
BF16 = mybir.dt.bfloat16
FP8 = mybir.dt.float8e4

_CACHE = {}


def _r3(ap, h, w):
    return ap.rearrange("p (h w) -> p h w", h=h, w=w)


def build_nc(m_reps=1, skip_ar=False, n_taps=9):
    nc = bacc.Bacc("TRN2", target_bir_lowering=False, debug=False, num_devices=NCORES)

    xbf_t = nc.dram_tensor("xbf", [BL, C, H, W], BF16, kind="ExternalInput")
    w_t = nc.dram_tensor("weight", [C, C, 3, 3], F32, kind="ExternalInput")
    g_t = nc.dram_tensor("gamma", [C], F32, kind="ExternalInput")
    b_t = nc.dram_tensor("beta", [C], F32, kind="ExternalInput")
    o_t = nc.dram_tensor("out", [BL, C, H, W], BF16, kind="ExternalOutput")

    ident_d = nc.inline_tensor(np.eye(P).astype(mybir.dt.np(BF16)), name="ident_bf16")

    with tile.TileContext(nc) as tc:
        with (
            tc.tile_pool(name="persist", bufs=1) as pp,
            tc.tile_pool(name="wnat", bufs=2) as wnat_pool,
            tc.tile_pool(name="wsign", bufs=2) as wsign_pool,
            tc.tile_pool(name="xin", bufs=8) as xin_pool,
            tc.tile_pool(name="t1", bufs=2) as t1_pool,
            tc.tile_pool(name="oput", bufs=3) as oput_pool,
            tc.tile_pool(name="dram", bufs=1, space="DRAM") as dram_pool,
        ):
            # persistent tiles
            ident = pp.tile([P, P], BF16, name="ident")
            wsb = pp.tile([P, CB, 9, CB, P], FP8, name="wsb")      # lhsT weights
            # padded sign(x)/2, 2-image ring as SEPARATE tiles so Tile's
            # whole-tile dependency tracking doesn't serialize sign(k+1)
            # against conv(k)'s reads of the other slot
            xbs = [
                pp.tile([P, CB, S2], FP8, name=f"xb{r}", uniquify=False)
                for r in range(2)
            ]
            ybuf = pp.tile([P, CB, BL, IMG], BF16, name="ybuf")    # conv output
            statsb = pp.tile([P, CB, BL * NT, 6], F32, name="statsb")
            gamma_sb = pp.tile([P, CB], F32, name="gamma_sb")
            beta_sb = pp.tile([P, CB], F32, name="beta_sb")
            loc_ms = pp.tile([P, CB, 2], F32, name="loc_ms")       # (mean,var) per cb
            ar_in = pp.tile([P, 4], F32, name="ar_in")
            g_stats = pp.tile([P, 4], F32, name="g_stats")
            s2_sb = pp.tile([P, CB], F32, name="s2_sb")
            bp_sb = pp.tile([P, CB], F32, name="bp_sb")
            gsc = pp.tile([P, 4], F32, name="gsc")
            gamma2_sb = pp.tile([P, CB], F32, name="gamma2_sb")
            tmp_a = pp.tile([P, CB], F32, name="tmp_a")
            tmp_b = pp.tile([P, CB], F32, name="tmp_b")
            tmp_c = pp.tile([P, CB], F32, name="tmp_c")
            eps_sb = pp.tile([P, 1], F32, name="eps_sb")
            warm_sb = pp.tile([P, 1], F32, name="warm_sb")
            nc.vector.memset(eps_sb[:], BN_EPS)
            # pre-load the sqrt activation table set off the critical path
            nc.scalar.activation(
                warm_sb[:], eps_sb[:], mybir.ActivationFunctionType.Sqrt,
                bias=eps_sb[:],
            )

            # zero the padded-input rings first (borders stay zero forever)
            for r in range(2):
                nc.vector.memset(xbs[r][:, :, :].bitcast(mybir.dt.uint32), 0)

            def emit_sign(img, eng, split_queues=False):
                # xb' = (x > 0) - 0.5 = sign(x)/2 — exact in fp8; the 2x is
                # folded into the BN sums and the phase-2 scale.
                ring = img % 2
                for cb in range(CB):
                    xin = xin_pool.tile([P, IMG], BF16, name="xin")
                    dma_eng = nc.gpsimd if (split_queues and cb == 1) else nc.sync
                    dma_eng.dma_start(
                        out=xin[:],
                        in_=xbf_t.ap()[img, cb * P : (cb + 1) * P, :, :].rearrange(
                            "p h w -> p (h w)"
                        ),
                    )
                    dst = _r3(xbs[ring][:, cb, PW + 1 : PW + 1 + H * PW], H, PW)
                    eng.tensor_scalar(
                        dst[:, :, :W],
                        _r3(xin[:], H, W),
                        0.0,
                        0.5,
                        mybir.AluOpType.is_gt,
                        mybir.AluOpType.subtract,
                    )

            # queue layout for the critical startup: cb_o=0 weights lead the
            # gpsimd queue (their transpose chain is the long pole for the
            # first conv group); image-0's two input DMAs take the sync
            # queue; cb_o=1 weights follow them there.
            def emit_wnat(cb_o, dma_eng):
                wnat = wnat_pool.tile([P, C * 9], F32, name="wnat")
                dma_eng.dma_start(
                    out=wnat[:],
                    in_=w_t.ap()[cb_o * P : (cb_o + 1) * P, :, :, :].rearrange(
                        "p a b c -> p (a b c)"
                    ),
                )
                return wnat

            emit_sign(0, nc.vector, split_queues=True)

            nc.sync.dma_start(out=ident[:], in_=ident_d.ap())
            with tc.tile_pool(name="psw", bufs=8, space="PSUM") as psw_pool:
                for cb_o in range(CB):
                    wnat = emit_wnat(cb_o, nc.gpsimd if cb_o == 0 else nc.sync)
                    wnat3 = wnat.rearrange("p (ci t) -> p ci t", t=9)
                    # binarize weights as (w>0)-0.5 = sign(w)/2 on DVE (keeps
                    # ScalarE free for the transpose drains); products are
                    # then y/4 — folded exactly into the BN sums and scale.
                    # Chunked by taps (separate tiles) so the first taps'
                    # lhsT weights are ready before the rest.
                    for t0, t1 in ((0, 3), (3, 6), (6, 9)):
                        wsgc = wsign_pool.tile([P, C, t1 - t0], BF16, name="wsig")
                        nc.gpsimd.tensor_scalar(
                            wsgc[:],
                            wnat3[:, :, t0:t1],
                            0.0,
                            0.5,
                            mybir.AluOpType.is_gt,
                            mybir.AluOpType.subtract,
                        )
                        for tap in range(t0, t1):
                            for cit in range(CB):
                                ptw = psw_pool.tile([P, P], BF16, name="ptw")
                                nc.tensor.transpose(
                                    ptw[:],
                                    wsgc[:, cit * P : (cit + 1) * P, tap - t0],
                                    ident[:],
                                )
                                # alternate drain engines to double the weight
                                # tile delivery rate to the conv stream
                                if cit == 0:
                                    nc.scalar.copy(wsb[:, cb_o, tap, cit, :], ptw[:])
                                else:
                                    nc.vector.tensor_copy(
                                        wsb[:, cb_o, tap, cit, :], ptw[:]
                                    )

            g2 = g_t.ap().rearrange("(a b) -> a b", b=1)
            b2 = b_t.ap().rearrange("(a b) -> a b", b=1)
            for cb in range(CB):
                nc.sync.dma_start(
                    out=gamma_sb[:, cb : cb + 1], in_=g2[cb * P : (cb + 1) * P, :]
                )
                nc.sync.dma_start(
                    out=beta_sb[:, cb : cb + 1], in_=b2[cb * P : (cb + 1) * P, :]
                )
            # 4*gamma precomputed off the critical path (y is stored as y/4)
            nc.scalar.mul(gamma2_sb[:, :], gamma_sb[:, :], 4.0)

            for _rep in range(m_reps):
                # ---- phase 1: sign(x), conv, drain+stats ----
                with tc.tile_pool(name="psc", bufs=8, space="PSUM") as psc_pool:
                    for img in range(BL):
                        ring = img % 2
                        if not (_rep == 0 and img == 0):
                            emit_sign(img, nc.vector)

                        for cb_o in range(CB):
                            lhs_all = wsb[:, cb_o, :, :, :]
                            for group in (range(0, 4), range(4, NT)):
                                pts = [
                                    psc_pool.tile([P, TIL], F32, name="ptc")
                                    for _ in group
                                ]
                                for tap in range(n_taps):
                                    kh, kw = tap // 3, tap % 3
                                    lhs = lhs_all[:, tap, :, :]
                                    for ti, t in enumerate(group):
                                        off = (t * 8 + kh) * PW + kw
                                        rhs = xbs[ring][:, :, off : off + TIL]
                                        nc.tensor.matmul(
                                            pts[ti][:],
                                            lhs,
                                            rhs,
                                            start=(tap == 0),
                                            stop=(tap == n_taps - 1),
                                            perf_mode=mybir.MatmulPerfMode.DoubleRow,
                                            skip_group_check=True,
                                        )
                                for ti, t in enumerate(group):
                                    pt3 = _r3(pts[ti][:], 8, PW)[:, :, :W]
                                    ysl = ybuf[:, cb_o, img, t * VAL : (t + 1) * VAL]
                                    nc.scalar.activation(
                                        _r3(ysl, 8, W), pt3,
                                        mybir.ActivationFunctionType.Copy,
                                    )
                                    # stats from the dense drained strip (no garbage)
                                    nc.vector.bn_stats(
                                        statsb[:, cb_o, img * NT + t, :], ysl
                                    )

                # prefetch all phase-2 residual inputs now — before anything
                # that waits on the collective lands in the DMA queues
                p2_xins = []
                for img in range(BL):
                    for cb in range(CB):
                        xin = xin_pool.tile([P, IMG], BF16, name="xin")
                        nc.sync.dma_start(
                            out=xin[:],
                            in_=xbf_t.ap()[img, cb * P : (cb + 1) * P, :, :].rearrange(
                                "p h w -> p (h w)"
                            ),
                        )
                        p2_xins.append(xin)

                # ---- local aggregation -> (sum, sumsq), per cb so cb0's
                # chain runs under cb1's conv tail.  Stats were computed on
                # y/4: sums scale by 4x and 16x exactly.
                ar2 = ar_in.rearrange("p (a b) -> p a b", b=2)
                for cb in range(CB):
                    nc.vector.bn_aggr(loc_ms[:, cb, :], statsb[:, cb, :, :])
                    m = loc_ms[:, cb, 0:1]
                    v = loc_ms[:, cb, 1:2]
                    nc.scalar.square(tmp_a[:, cb : cb + 1], m)
                    nc.vector.tensor_add(tmp_b[:, cb : cb + 1], v, tmp_a[:, cb : cb + 1])
                    nc.scalar.mul(ar2[:, cb, 0:1], m, float(4 * N_LOC))
                    nc.scalar.mul(
                        ar2[:, cb, 1:2], tmp_b[:, cb : cb + 1], float(16 * N_LOC)
                    )

                cci = dram_pool.tile([P, 4], F32, name="cc_in")
                cco = dram_pool.tile([P, 4], F32, name="cc_out")
                nc.sync.dma_start(out=cci[:], in_=ar_in[:])
                if skip_ar:
                    nc.sync.dma_start(out=cco[:], in_=cci[:])
                else:
                    nc.gpsimd.collective_compute(
                        "AllReduce",
                        mybir.AluOpType.add,
                        replica_groups=[list(range(NCORES))],
                        ins=[cci.opt()],
                        outs=[cco.opt()],
                    )
                # readback on the sync queue is safe now: every DMA emitted
                # after it (the phase-2 stores) is collective-dependent anyway
                nc.sync.dma_start(out=g_stats[:], in_=cco[:])

                # ---- global scale/bias: s2 = 4*gamma*rsqrt(var+eps) (stored y
                # is y/4), b' = beta - mean*s.  Minimal post-collective chain.
                inv_n = 1.0 / float(N_GLOB)
                nc.scalar.mul(gsc[:, :], g_stats[:, :], inv_n)     # means+ex2 at once
                gs2 = gsc.rearrange("p (a b) -> p a b", b=2)
                gmean = gs2[:, :, 0]
                gex2 = gs2[:, :, 1]
                nc.scalar.square(tmp_a[:, :], gmean)
                nc.vector.tensor_sub(tmp_b[:, :], gex2, tmp_a[:, :])   # var
                nc.scalar.activation(
                    tmp_c[:, :], tmp_b[:, :], mybir.ActivationFunctionType.Sqrt,
                    bias=eps_sb[:],
                )
                nc.vector.reciprocal(tmp_a[:, :], tmp_c[:, :])     # 1/sqrt(var+eps)
                nc.vector.tensor_mul(s2_sb[:, :], tmp_a[:, :], gamma2_sb[:, :])
                nc.vector.tensor_mul(tmp_b[:, :], gmean, s2_sb[:, :])  # 4*mean*s
                # b' = beta - 0.25*(4*mean*s)
                nc.vector.scalar_tensor_tensor(
                    bp_sb[:, :], tmp_b[:, :], -0.25, beta_sb[:, :],
                    mybir.AluOpType.mult, mybir.AluOpType.add,
                )

                # ---- phase 2: out = y*s + b' + x (vector-only pipeline) ----
                for img in range(BL):
                    for cb in range(CB):
                        k = img * CB + cb
                        xin = p2_xins[k]
                        t1 = t1_pool.tile([P, IMG], BF16, name="t1")
                        if k in (1, 3, 5):
                            # run some scale+bias passes on the otherwise-idle
                            # ScalarE so the DVE pipe only carries the adds
                            nc.scalar.activation(
                                t1[:],
                                ybuf[:, cb, img, :],
                                mybir.ActivationFunctionType.Identity,
                                bias=bp_sb[:, cb : cb + 1],
                                scale=s2_sb[:, cb : cb + 1],
                            )
                        else:
                            nc.vector.tensor_scalar(
                                t1[:],
                                ybuf[:, cb, img, :],
                                s2_sb[:, cb : cb + 1],
                                bp_sb[:, cb : cb + 1],
                                mybir.AluOpType.mult,
                                mybir.AluOpType.add,
                            )
                        oput = oput_pool.tile([P, IMG], BF16, name="oput")
                        nc.vector.tensor_add(oput[:], t1[:], xin[:])
                        nc.sync.dma_start(
                            out=o_t.ap()[img, cb * P : (cb + 1) * P, :, :].rearrange(
                                "p h w -> p (h w)"
                            ),
                            in_=oput[:],
                        )

    nc.compile()
    return nc


def get_nc():
    if "nc" not in _CACHE:
        _CACHE["nc"] = build_nc()
    return _CACHE["nc"]


def kernel(x, weight, gamma, beta, _want_results=False, **run_kwargs):
    nc = get_nc()
    bf16 = mybir.dt.np(BF16)
    in_maps = []
    for core in range(NCORES):
        shard = np.ascontiguousarray(x[core * BL : (core + 1) * BL]).astype(
            np.float32, copy=False
        )
        in_maps.append(
            {
                "xbf": shard.astype(bf16),
                "weight": np.ascontiguousarray(weight).astype(np.float32, copy=False),
                "gamma": np.ascontiguousarray(gamma).astype(np.float32, copy=False),
                "beta": np.ascontiguousarray(beta).astype(np.float32, copy=False),
            }
        )
    res = run_bass_kernel_spmd(nc, in_maps, core_ids=list(range(NCORES)), **run_kwargs)
    out = np.concatenate(
        [
            np.asarray(res.results[c]["out"])
            .astype(np.float32)
            .reshape(BL, C, H, W)
            for c in range(NCORES)
        ],
        axis=0,
    )
    if _want_results:
        return out, res
    return out
